# revision 27
# baseline (speedup 1.0000x reference)
"""Differential attention kernel for Trainium2, 8-core SPMD.

Problem (hardcoded shapes): B=2, S=2048, D=2048, H=16 heads, head_dim=128,
dual-chunk q/k dim 64.  out = (softmax(q1k1*s+m) - lam*softmax(q2k2*s+m)) @ v,
then output projection.

Sharding: batch x head-group.  Core c handles batch c//4 and heads
(c%4)*4 .. +4.  Each core computes its 4 heads' QKV columns (tensor
parallel on c_attn output cols), full attention for those heads, and a
partial output projection (tensor parallel on c_proj input rows).  The
4 partial projections per batch are summed on host (the "unshard").

All matmul operands are bf16 end-to-end (x/W inputs converted on host,
q/k/v/exp/g tiles on chip): bf16 moving operands issue at 216 ns per
512-row matmul vs 227 ns for float32r, stationary loads halve, and
input DMA halves.  Total error ~5e-3 vs the 2e-2 gate (fp8 variants
were measured over the gate and rejected).

Layout strategy (everything "transposed" so contraction dims land on
SBUF partitions, no on-chip transposes anywhere):
  - host passes X^T [D, S] per batch
  - QKV phase produces, SBUF-resident: per head h a packed bf16 Q^T tile
    q12[h] [128, S] (rows 0:64 = q1^T, 64:128 = q2^T), ditto k12[h],
    and bf16 V in natural layout v_h[h] [128, KC, 128] (partition = s%128).
  - scores computed transposed per head/q-tile: s^T [k, q] psum tiles;
    the two dual-chunk K=64 bf16 score matmuls are row-group packed
    (rows 0:63 / 64:127) and measured to stream concurrently at ~2
    rows/cycle (a pair of 512-row matmuls issues in ~230 ns);
  - exp on ScalarE (scale + a constant -1.5 bias folded in; attn mask
    bias folded in when nontrivial) at [128, 1024] granularity straight
    out of PSUM into bf16 e-tiles [128, 2(chain), 2(kc), 512];
  - softmax denominators via all-ones [128,128] stationary matmuls that
    produce the denominator PRE-BROADCAST over all 128 partitions (d
    psum [128,512] per chain), feeding a fast approximate reciprocal
    (reciprocal_approx_fast, ~18 bits) directly at [128,512] - no
    gpsimd partition_broadcast and no slow 1-partition reciprocal;
  - PV + denominator chains trail the scores/exp stream by one k-pair
    (software pipeline) so the PE never waits on ScalarE;
  - PV gives O^T [hd, q] which is exactly the lhsT layout the output
    projection wants; normalization is a fused multiply / multiply-sub
    on the VectorE writing bf16 g tiles.

Engine balance per core (measured): PE ~505-590 us busy of ~650 us,
ScalarE exp 285 us, VectorE ~130 us.  The kernel is PE-throughput
bound; fp8 DoubleRow (2x PE) was implemented and measured correct on
the denominator path but the required e/v quantization exceeds the
2e-2 error gate, and an fp8 side-copy for denominators-only lost more
to cast bandwidth than the PE saved.
"""

import ml_dtypes
import numpy as np

import concourse.bass as bass
import concourse.mybir as mybir
import concourse.tile as tile
from concourse import bacc
from concourse.bass_utils import run_bass_kernel_spmd

F32 = mybir.dt.float32
F32R = mybir.dt.float32r
BF16 = mybir.dt.bfloat16
FP8 = mybir.dt.float8e4
EXP = mybir.ActivationFunctionType.Exp
MULT = mybir.AluOpType.mult
DR = mybir.MatmulPerfMode.DoubleRow

B, S, D, H = 2, 2048, 2048, 16
HD = D // H            # 128 full head dim
QD = HD // 2           # 64 dual-chunk q/k dim
N_CORES = 8
HPC = H // (N_CORES // B)   # 4 heads per core
CPB = N_CORES // B          # 4 cores per batch
SCALE = float(HD) ** -0.5
EBIAS = -1.5          # exp(s*scale + EBIAS): cancels in softmax,
                      # keeps exp outputs in fp8-castable range
KC = S // 128          # 16 k-chunks
DC = D // 128          # 16 d-chunks
SCW = 512              # QKV s-chunk width
XSUB = 4               # d-chunks per xt subtile


def build_program(lam: float, mask_trivial: bool):
    nc = bacc.Bacc("TRN2", target_bir_lowering=False, debug=False,
                   enable_asserts=False, num_devices=N_CORES)

    xt = nc.dram_tensor("xt", [D, S], BF16, kind="ExternalInput").ap()
    wqk = nc.dram_tensor("wqk", [D, 2 * HPC * HD], BF16, kind="ExternalInput").ap()
    wv = nc.dram_tensor("wv", [D, HPC * HD], BF16, kind="ExternalInput").ap()
    wp = nc.dram_tensor("wp", [HPC * HD, D], BF16, kind="ExternalInput").ap()
    maskb = nc.dram_tensor("maskb", [KC, 128], F32, kind="ExternalInput").ap()
    y = nc.dram_tensor("y", [S, D], F32, kind="ExternalOutput").ap()

    NQK = 2 * HPC  # 8 qk c-tiles of 128
    with tile.TileContext(nc) as tc:
        with (
            tc.tile_pool(name="consts", bufs=1) as cpool,
            tc.tile_pool(name="qkvres", bufs=1) as qkvpool,
        ):
            mask_t = cpool.tile([128, KC], F32, tag="mask")
            nc.sync.dma_start(mask_t[:], maskb.rearrange("c p -> p c"))

            q12, k12, v_h = [], [], []
            for h in range(HPC):
                q12.append(qkvpool.tile([128, S], BF16, tag=f"q{h}",
                                        name=f"q12_{h}"))
                k12.append(qkvpool.tile([128, S], BF16, tag=f"k{h}",
                                        name=f"k12_{h}"))
                v_h.append(qkvpool.tile([128, KC, HD], BF16, tag=f"v{h}",
                                        name=f"v_{h}"))
            # all-ones [128, 128] stationary for the broadcast
            # denominator matmuls (out row m = sum_k e[k, :] for every m)
            dones = cpool.tile([128, 128], BF16, tag="dones")
            nc.gpsimd.memset(dones[:], 1.0)
            ebias_t = cpool.tile([128, 1], F32, tag="ebias")
            nc.gpsimd.memset(ebias_t[:], EBIAS)

            # ---------------- Phase 1: QKV projections ----------------
            with (
                tc.tile_pool(name="xtp", bufs=6) as xtp,
                tc.tile_pool(name="wqkp", bufs=3) as wqkp,
                tc.tile_pool(name="wvp", bufs=1) as wvp,
                tc.tile_pool(name="qkv_ev", bufs=3) as evp,
                tc.tile_pool(name="qkv_ps", bufs=4, space="PSUM") as psp,
            ):
                wv_t = wvp.tile([128, DC, HPC * HD], BF16, tag="wv")
                ct_order = [4, 0, 5, 1, 6, 2, 7, 3]
                w_cache: dict = {}

                def load_w(ct):
                    w = wqkp.tile([128, DC, 128], BF16, tag="wqk",
                                  name=f"wqk_{ct}")
                    nc.sync.dma_start(
                        w[:],
                        wqk[:, ct * 128:(ct + 1) * 128]
                        .rearrange("(c p) m -> p c m", p=128))
                    return w

                for sc in range(S // SCW):
                    ss = slice(sc * SCW, (sc + 1) * SCW)
                    xt_sub = []
                    for i in range(DC // XSUB):
                        t = xtp.tile([128, XSUB, SCW], BF16, tag="xt",
                                     name=f"xt_{sc}_{i}")
                        nc.sync.dma_start(
                            t[:],
                            xt[i * XSUB * 128:(i + 1) * XSUB * 128, ss]
                            .rearrange("(c p) s -> p c s", p=128))
                        xt_sub.append(t)
                        if sc == 0:
                            # interleave the wv slices with the xt subtiles
                            # so the first V matmul chain starts early
                            sl = slice(i * XSUB, (i + 1) * XSUB)
                            nc.sync.dma_start(
                                wv_t[:, sl, :],
                                wv.rearrange("(c p) n -> p c n", p=128)[:, sl, :])

                    def xtc(dc):
                        return xt_sub[dc // XSUB][:, dc % XSUB, :]

                    # snake the c-tile order so pool-resident weight tiles
                    # from the previous s-chunk get reused at the boundary
                    order = ct_order if sc % 2 == 0 else ct_order[::-1]

                    # V (natural layout): lhsT = X^T chunk, rhs = Wv
                    for st in range(SCW // 128):
                        ps = psp.tile([128, 512], F32, tag="ps")
                        for dc in range(DC):
                            nc.tensor.matmul(
                                ps[:],
                                xtc(dc)[:, st * 128:(st + 1) * 128],
                                wv_t[:, dc, :],
                                start=(dc == 0), stop=(dc == DC - 1))
                        stg = sc * (SCW // 128) + st
                        for h in range(HPC):
                            nc.vector.tensor_copy(
                                v_h[h][:, stg, :],
                                ps[:, h * HD:(h + 1) * HD])

                    # Q^T / K^T c-tiles (k first so attention unblocks early)
                    for ct in order:
                        w_t = w_cache.pop(ct, None)
                        if w_t is None:
                            w_t = load_w(ct)
                        ps = psp.tile([128, 512], F32, tag="ps")
                        for dc in range(DC):
                            nc.tensor.matmul(
                                ps[:], w_t[:, dc, :], xtc(dc),
                                start=(dc == 0), stop=(dc == DC - 1))
                        dst = k12[ct - HPC] if ct >= HPC else q12[ct]
                        nc.vector.tensor_copy(dst[:, ss], ps[:])
                        last_w = (ct, w_t)
                    # only the most recent tile survives the pool rotation
                    w_cache = {last_w[0]: last_w[1]}

            # ---------------- Phase 2: differential attention ----------------
            gpool_cm = tc.tile_pool(name="gbuf", bufs=1)
            gpool = gpool_cm.__enter__()
            g_tiles = []
            for h in range(HPC):
                g_tiles.append(
                    gpool.tile([HD, S], BF16, tag=f"g{h}", name=f"g{h}"))
            wpp_cm = tc.tile_pool(name="wpp", bufs=1)
            wpp = wpp_cm.__enter__()
            wp_tiles: list = []

            def load_wp(h):
                w = wpp.tile([HD, D], BF16, tag=f"wp{h}", name=f"wp{h}")
                nc.sync.dma_start(
                    w[:], wp[h * HD:(h + 1) * HD, :])
                wp_tiles.append(w)

            with (
                tc.tile_pool(name="e12p", bufs=4) as epool,
                tc.tile_pool(name="rp", bufs=2) as rpool,
                tc.tile_pool(name="tp", bufs=1) as tpool,
                tc.tile_pool(name="att_s", bufs=2, space="PSUM") as spsum,
                tc.tile_pool(name="att_o", bufs=2, space="PSUM") as opsum,
                tc.tile_pool(name="att_d", bufs=2, space="PSUM") as dpsum,
            ):
                for h in range(HPC):
                    if h == HPC - 1:
                        # DMA is idle during attention; stage the first
                        # projection weights before the phase boundary
                        load_wp(0)
                        load_wp(1)
                    for qt in range(S // 512):
                        qs = slice(qt * 512, (qt + 1) * 512)
                        o1 = opsum.tile([128, 512], F32, tag="o")
                        o2 = opsum.tile([128, 512], F32, tag="o")
                        d1 = dpsum.tile([128, 512], F32, tag="d")
                        d2 = dpsum.tile([128, 512], F32, tag="d")
                        e_pending = None  # software pipeline: scores/exp of
                        # pair pg are emitted before denom/PV of pair pg-1 so
                        # the PE never has to sit out an exp

                        def emit_dpv(pg, e12):
                            first = (pg == 0)
                            last = (pg == KC // 2 - 1)
                            for j in range(2):
                                kc = 2 * pg + j
                                f = first and j == 0
                                l = last and j == 1
                                nc.tensor.matmul(o1[:], v_h[h][:, kc, :],
                                                 e12[:, 0, j, :],
                                                 start=f, stop=l)
                                nc.tensor.matmul(d1[:], dones[:],
                                                 e12[:, 0, j, :],
                                                 start=f, stop=l)
                                nc.tensor.matmul(o2[:], v_h[h][:, kc, :],
                                                 e12[:, 1, j, :],
                                                 start=f, stop=l)
                                nc.tensor.matmul(d2[:], dones[:],
                                                 e12[:, 1, j, :],
                                                 start=f, stop=l)

                        for pg in range(KC // 2):
                            s1p = spsum.tile([128, 1024], F32, tag="s")
                            s2p = spsum.tile([128, 1024], F32, tag="s")
                            for j in range(2):
                                kc = pg * 2 + j
                                js = slice(j * 512, (j + 1) * 512)
                                nc.tensor.matmul(
                                    s1p[:, js],
                                    k12[h][0:QD, kc * 128:(kc + 1) * 128],
                                    q12[h][0:QD, qs], start=True, stop=True)
                                nc.tensor.matmul(
                                    s2p[:, js],
                                    k12[h][QD:128, kc * 128:(kc + 1) * 128],
                                    q12[h][QD:128, qs], start=True, stop=True)
                            # e12[:, c, j, :] = exp of chain c, k-chunk 2pg+j
                            e12 = epool.tile([128, 2, 2, 512], BF16,
                                             tag="e12")
                            if mask_trivial:
                                nc.scalar.activation(e12[:, 0, :, :], s1p[:],
                                                     EXP, bias=ebias_t[:],
                                                     scale=SCALE)
                                nc.scalar.activation(e12[:, 1, :, :], s2p[:],
                                                     EXP, bias=ebias_t[:],
                                                     scale=SCALE)
                            else:
                                for j in range(2):
                                    kc = pg * 2 + j
                                    js = slice(j * 512, (j + 1) * 512)
                                    mb = mask_t[:, kc:kc + 1]
                                    nc.scalar.activation(e12[:, 0, j, :],
                                                         s1p[:, js],
                                                         EXP, bias=mb,
                                                         scale=SCALE)
                                    nc.scalar.activation(e12[:, 1, j, :],
                                                         s2p[:, js],
                                                         EXP, bias=mb,
                                                         scale=SCALE)
                            if e_pending is not None:
                                emit_dpv(pg - 1, e_pending)
                            e_pending = e12
                        emit_dpv(KC // 2 - 1, e_pending)

                        R1 = rpool.tile([128, 512], F32, tag="r")
                        nc.vector.reciprocal_approx_fast(R1[:], d1[:])
                        R2 = rpool.tile([128, 512], F32, tag="r")
                        nc.vector.reciprocal_approx_fast(R2[:], d2[:])

                        u1 = tpool.tile([128, 512], F32, tag="u1")
                        nc.vector.tensor_mul(u1[:], o1[:], R1[:])
                        u2 = tpool.tile([128, 512], F32, tag="u2")
                        nc.vector.scalar_tensor_tensor(
                            u2[:], o2[:], lam, R2[:], op0=MULT, op1=MULT)
                        nc.vector.tensor_sub(g_tiles[h][:, qs], u1[:], u2[:])
            # ---------------- Phase 3: output projection ----------------
            with (
                tc.tile_pool(name="wpp2", bufs=1) as wpp2,
                tc.tile_pool(name="yev", bufs=3) as yev,
                tc.tile_pool(name="proj_ps", bufs=4, space="PSUM") as ppsum,
            ):
                for h in (2, 3):
                    w = wpp2.tile([HD, D], BF16, tag=f"wp{h}", name=f"wp{h}")
                    nc.sync.dma_start(
                        w[:], wp[h * HD:(h + 1) * HD, :])
                    wp_tiles.append(w)
                for st in range(S // 128):
                    yt = yev.tile([128, D], F32, tag="yt")
                    for et in range(D // 512):
                        ps = ppsum.tile([128, 512], F32, tag="ps")
                        for h in range(HPC):
                            nc.tensor.matmul(
                                ps[:],
                                g_tiles[h][:, st * 128:(st + 1) * 128],
                                wp_tiles[h][:, et * 512:(et + 1) * 512],
                                start=(h == 0), stop=(h == HPC - 1))
                        nc.vector.tensor_copy(yt[:, et * 512:(et + 1) * 512],
                                              ps[:])
                    nc.sync.dma_start(y[st * 128:(st + 1) * 128, :], yt[:])
            wpp_cm.__exit__(None, None, None)
            gpool_cm.__exit__(None, None, None)

    nc.compile()
    return nc


_PROGRAM_CACHE: dict = {}


def _get_program(lam: float, mask_trivial: bool):
    key = (round(lam, 9), mask_trivial)
    if key not in _PROGRAM_CACHE:
        _PROGRAM_CACHE[key] = build_program(lam, mask_trivial)
    return _PROGRAM_CACHE[key]


def make_in_maps(hidden_states, attention_mask, W_attn, b_attn, W_proj):
    in_maps = []
    for c in range(N_CORES):
        b = c // CPB
        h0 = (c % CPB) * HPC
        xt = np.ascontiguousarray(hidden_states[b].T)
        cols = []
        for h in range(h0, h0 + HPC):
            cols.append(W_attn[:, h * QD:(h + 1) * QD])              # q1
            cols.append(W_attn[:, D // 2 + h * QD:D // 2 + (h + 1) * QD])  # q2
        for h in range(h0, h0 + HPC):
            cols.append(W_attn[:, D + h * QD:D + (h + 1) * QD])      # k1
            cols.append(W_attn[:, D + D // 2 + h * QD:D + D // 2 + (h + 1) * QD])
        wqk = np.ascontiguousarray(np.concatenate(cols, axis=1))
        wv = np.ascontiguousarray(W_attn[:, 2 * D + h0 * HD:2 * D + (h0 + HPC) * HD])
        wpm = np.ascontiguousarray(W_proj[h0 * HD:(h0 + HPC) * HD, :])
        maskb = np.ascontiguousarray(
            ((1.0 - attention_mask[b]) * -10000.0 + EBIAS).reshape(KC, 128)
        ).astype(np.float32)
        in_maps.append({
            "xt": xt.astype(ml_dtypes.bfloat16),
            "wqk": wqk.astype(ml_dtypes.bfloat16),
            "wv": wv.astype(ml_dtypes.bfloat16),
            "wp": wpm.astype(ml_dtypes.bfloat16),
            "maskb": maskb,
        })
    return in_maps


def kernel(hidden_states, attention_mask, W_attn, b_attn, W_proj, b_proj,
           lambda_param, _trace=False):
    hidden_states = np.asarray(hidden_states, np.float32)
    attention_mask = np.asarray(attention_mask, np.float32)
    W_attn = np.asarray(W_attn, np.float32)
    b_attn = np.asarray(b_attn, np.float32)
    W_proj = np.asarray(W_proj, np.float32)
    b_proj = np.asarray(b_proj, np.float32)
    lam = float(np.asarray(lambda_param))

    if np.any(b_attn != 0.0):
        raise NotImplementedError("nonzero b_attn not supported")

    mask_trivial = bool(np.all(attention_mask == 1.0))
    nc = _get_program(lam, mask_trivial)
    in_maps = make_in_maps(hidden_states, attention_mask, W_attn, b_attn,
                           W_proj)
    try:
        res = run_bass_kernel_spmd(nc, in_maps, core_ids=list(range(N_CORES)),
                                   trace=_trace)
    except ModuleNotFoundError:
        res = run_bass_kernel_spmd(nc, in_maps, core_ids=list(range(N_CORES)),
                                   trace=False)

    out = np.empty((B, S, D), np.float32)
    for b in range(B):
        acc = res.results[b * CPB]["y"].astype(np.float32).copy()
        for c in range(b * CPB + 1, (b + 1) * CPB):
            acc += res.results[c]["y"]
        out[b] = acc + b_proj[None, :]
    kernel.last_exec_time_ns = res.exec_time_ns
    if res.instructions_and_trace is not None:
        kernel.last_trace_path = res.instructions_and_trace[1]
    return out


kernel.last_exec_time_ns = None
kernel.last_trace_path = None



# revision 29
# speedup vs baseline: 1.0103x; 1.0103x over previous
"""Differential attention kernel for Trainium2, 8-core SPMD.

Problem (hardcoded shapes): B=2, S=2048, D=2048, H=16 heads, head_dim=128,
dual-chunk q/k dim 64.  out = (softmax(q1k1*s+m) - lam*softmax(q2k2*s+m)) @ v,
then output projection.

Sharding: batch x head-group.  Core c handles batch c//4 and heads
(c%4)*4 .. +4.  Each core computes its 4 heads' QKV columns (tensor
parallel on c_attn output cols), full attention for those heads, and a
partial output projection (tensor parallel on c_proj input rows).  The
4 partial projections per batch are summed on host (the "unshard").

All matmul operands are bf16 end-to-end (x/W inputs converted on host,
q/k/v/exp/g tiles on chip): bf16 moving operands issue at 216 ns per
512-row matmul vs 227 ns for float32r, stationary loads halve, and
input DMA halves.  Total error ~5e-3 vs the 2e-2 gate (fp8 variants
were measured over the gate and rejected).

Layout strategy (everything "transposed" so contraction dims land on
SBUF partitions, no on-chip transposes anywhere):
  - host passes X^T [D, S] per batch
  - QKV phase produces, SBUF-resident: per head h a packed bf16 Q^T tile
    q12[h] [128, S] (rows 0:64 = q1^T, 64:128 = q2^T), ditto k12[h],
    and all-head bf16 V in one tile v_all [128, KC, 4*128]
    (partition = s%128) so each QKV psum drains in one [128,512] copy.
  - scores computed transposed per head/q-tile: s^T [k, q] psum tiles;
    the two dual-chunk K=64 bf16 score matmuls are row-group packed
    (rows 0:63 / 64:127) and measured to stream concurrently at ~2
    rows/cycle (a pair of 512-row matmuls issues in ~230 ns);
  - exp on ScalarE (scale + a constant -1.5 bias folded in; attn mask
    bias folded in when nontrivial) at [128, 1024] granularity straight
    out of PSUM into bf16 e-tiles [128, 2(chain), 2(kc), 512];
  - softmax denominators via all-ones [128,128] stationary matmuls that
    produce the denominator PRE-BROADCAST over all 128 partitions (d
    psum [128,512] per chain), feeding a fast approximate reciprocal
    (reciprocal_approx_fast, ~18 bits) directly at [128,512] - no
    gpsimd partition_broadcast and no slow 1-partition reciprocal;
  - PV + denominator chains trail the scores/exp stream by one k-pair
    (software pipeline) so the PE never waits on ScalarE;
  - PV gives O^T [hd, q] which is exactly the lhsT layout the output
    projection wants; normalization is a fused multiply / multiply-sub
    on the VectorE writing bf16 g tiles.

Engine balance per core (measured): PE ~505-590 us busy of ~650 us,
ScalarE exp 285 us, VectorE ~130 us.  The kernel is PE-throughput
bound; fp8 DoubleRow (2x PE) was implemented and measured correct on
the denominator path but the required e/v quantization exceeds the
2e-2 error gate, and an fp8 side-copy for denominators-only lost more
to cast bandwidth than the PE saved.
"""

import ml_dtypes
import numpy as np

import concourse.bass as bass
import concourse.mybir as mybir
import concourse.tile as tile
from concourse import bacc
from concourse.bass_utils import run_bass_kernel_spmd

F32 = mybir.dt.float32
F32R = mybir.dt.float32r
BF16 = mybir.dt.bfloat16
FP8 = mybir.dt.float8e4
EXP = mybir.ActivationFunctionType.Exp
MULT = mybir.AluOpType.mult
DR = mybir.MatmulPerfMode.DoubleRow

B, S, D, H = 2, 2048, 2048, 16
HD = D // H            # 128 full head dim
QD = HD // 2           # 64 dual-chunk q/k dim
N_CORES = 8
HPC = H // (N_CORES // B)   # 4 heads per core
CPB = N_CORES // B          # 4 cores per batch
SCALE = float(HD) ** -0.5
EBIAS = -1.5          # exp(s*scale + EBIAS): cancels in softmax,
                      # keeps exp outputs in fp8-castable range
KC = S // 128          # 16 k-chunks
DC = D // 128          # 16 d-chunks
SCW = 512              # QKV s-chunk width
XSUB = 4               # d-chunks per xt subtile


def build_program(lam: float, mask_trivial: bool):
    nc = bacc.Bacc("TRN2", target_bir_lowering=False, debug=False,
                   enable_asserts=False, num_devices=N_CORES)

    xt = nc.dram_tensor("xt", [D, S], BF16, kind="ExternalInput").ap()
    wqk = nc.dram_tensor("wqk", [D, 2 * HPC * HD], BF16, kind="ExternalInput").ap()
    wv = nc.dram_tensor("wv", [D, HPC * HD], BF16, kind="ExternalInput").ap()
    wp = nc.dram_tensor("wp", [HPC * HD, D], BF16, kind="ExternalInput").ap()
    maskb = nc.dram_tensor("maskb", [KC, 128], F32, kind="ExternalInput").ap()
    y = nc.dram_tensor("y", [S, D], F32, kind="ExternalOutput").ap()

    NQK = 2 * HPC  # 8 qk c-tiles of 128
    with tile.TileContext(nc) as tc:
        with (
            tc.tile_pool(name="consts", bufs=1) as cpool,
            tc.tile_pool(name="qkvres", bufs=1) as qkvpool,
        ):
            mask_t = cpool.tile([128, KC], F32, tag="mask")
            nc.sync.dma_start(mask_t[:], maskb.rearrange("c p -> p c"))

            q12, k12 = [], []
            for h in range(HPC):
                q12.append(qkvpool.tile([128, S], BF16, tag=f"q{h}",
                                        name=f"q12_{h}"))
                k12.append(qkvpool.tile([128, S], BF16, tag=f"k{h}",
                                        name=f"k12_{h}"))
            # all heads' V in one tile so each QKV psum drains in a single
            # [128,512] copy; head h's chunk kc is v_all[:, kc, h*HD:...]
            v_all = qkvpool.tile([128, KC, HPC * HD], BF16, tag="v_all")
            # all-ones [128, 128] stationary for the broadcast
            # denominator matmuls (out row m = sum_k e[k, :] for every m)
            dones = cpool.tile([128, 128], BF16, tag="dones")
            nc.gpsimd.memset(dones[:], 1.0)
            ebias_t = cpool.tile([128, 1], F32, tag="ebias")
            nc.gpsimd.memset(ebias_t[:], EBIAS)

            # ---------------- Phase 1: QKV projections ----------------
            with (
                tc.tile_pool(name="xtp", bufs=6) as xtp,
                tc.tile_pool(name="wqkp", bufs=3) as wqkp,
                tc.tile_pool(name="wvp", bufs=1) as wvp,
                tc.tile_pool(name="qkv_ev", bufs=3) as evp,
                tc.tile_pool(name="qkv_ps", bufs=5, space="PSUM") as psp,
            ):
                wv_t = wvp.tile([128, DC, HPC * HD], BF16, tag="wv")
                ct_order = [4, 0, 5, 1, 6, 2, 7, 3]
                w_cache: dict = {}

                def load_w(ct):
                    w = wqkp.tile([128, DC, 128], BF16, tag="wqk",
                                  name=f"wqk_{ct}")
                    nc.sync.dma_start(
                        w[:],
                        wqk[:, ct * 128:(ct + 1) * 128]
                        .rearrange("(c p) m -> p c m", p=128))
                    return w

                for sc in range(S // SCW):
                    ss = slice(sc * SCW, (sc + 1) * SCW)
                    xt_sub = []
                    for i in range(DC // XSUB):
                        t = xtp.tile([128, XSUB, SCW], BF16, tag="xt",
                                     name=f"xt_{sc}_{i}")
                        nc.sync.dma_start(
                            t[:],
                            xt[i * XSUB * 128:(i + 1) * XSUB * 128, ss]
                            .rearrange("(c p) s -> p c s", p=128))
                        xt_sub.append(t)
                        if sc == 0:
                            # interleave the wv slices with the xt subtiles
                            # so the first V matmul chain starts early
                            sl = slice(i * XSUB, (i + 1) * XSUB)
                            nc.sync.dma_start(
                                wv_t[:, sl, :],
                                wv.rearrange("(c p) n -> p c n", p=128)[:, sl, :])

                    def xtc(dc):
                        return xt_sub[dc // XSUB][:, dc % XSUB, :]

                    # snake the c-tile order so pool-resident weight tiles
                    # from the previous s-chunk get reused at the boundary
                    order = ct_order if sc % 2 == 0 else ct_order[::-1]

                    # V (natural layout): lhsT = X^T chunk, rhs = Wv
                    for st in range(SCW // 128):
                        ps = psp.tile([128, 512], F32, tag="ps")
                        for dc in range(DC):
                            nc.tensor.matmul(
                                ps[:],
                                xtc(dc)[:, st * 128:(st + 1) * 128],
                                wv_t[:, dc, :],
                                start=(dc == 0), stop=(dc == DC - 1))
                        stg = sc * (SCW // 128) + st
                        nc.vector.tensor_copy(v_all[:, stg, :], ps[:])

                    # Q^T / K^T c-tiles (k first so attention unblocks early)
                    for ct in order:
                        w_t = w_cache.pop(ct, None)
                        if w_t is None:
                            w_t = load_w(ct)
                        ps = psp.tile([128, 512], F32, tag="ps")
                        for dc in range(DC):
                            nc.tensor.matmul(
                                ps[:], w_t[:, dc, :], xtc(dc),
                                start=(dc == 0), stop=(dc == DC - 1))
                        dst = k12[ct - HPC] if ct >= HPC else q12[ct]
                        nc.vector.tensor_copy(dst[:, ss], ps[:])
                        last_w = (ct, w_t)
                    # only the most recent tile survives the pool rotation
                    w_cache = {last_w[0]: last_w[1]}

            # ---------------- Phase 2: differential attention ----------------
            gpool_cm = tc.tile_pool(name="gbuf", bufs=1)
            gpool = gpool_cm.__enter__()
            g_tiles = []
            for h in range(HPC):
                g_tiles.append(
                    gpool.tile([HD, S], BF16, tag=f"g{h}", name=f"g{h}"))
            wpp_cm = tc.tile_pool(name="wpp", bufs=1)
            wpp = wpp_cm.__enter__()
            wp_tiles: list = []

            def load_wp(h):
                w = wpp.tile([HD, D], BF16, tag=f"wp{h}", name=f"wp{h}")
                nc.sync.dma_start(
                    w[:], wp[h * HD:(h + 1) * HD, :])
                wp_tiles.append(w)

            with (
                tc.tile_pool(name="e12p", bufs=4) as epool,
                tc.tile_pool(name="rp", bufs=2) as rpool,
                tc.tile_pool(name="tp", bufs=1) as tpool,
                tc.tile_pool(name="att_s", bufs=2, space="PSUM") as spsum,
                tc.tile_pool(name="att_o", bufs=2, space="PSUM") as opsum,
                tc.tile_pool(name="att_d", bufs=2, space="PSUM") as dpsum,
            ):
                for h in range(HPC):
                    if h == HPC - 1:
                        # DMA is idle during attention; stage the first
                        # projection weights before the phase boundary
                        load_wp(0)
                        load_wp(1)
                    for qt in range(S // 512):
                        qs = slice(qt * 512, (qt + 1) * 512)
                        o1 = opsum.tile([128, 512], F32, tag="o")
                        o2 = opsum.tile([128, 512], F32, tag="o")
                        d1 = dpsum.tile([128, 512], F32, tag="d")
                        d2 = dpsum.tile([128, 512], F32, tag="d")
                        e_pending = None  # software pipeline: scores/exp of
                        # pair pg are emitted before denom/PV of pair pg-1 so
                        # the PE never has to sit out an exp

                        def emit_dpv(pg, e12):
                            first = (pg == 0)
                            last = (pg == KC // 2 - 1)
                            for j in range(2):
                                kc = 2 * pg + j
                                f = first and j == 0
                                l = last and j == 1
                                vv = v_all[:, kc,
                                           h * HD:(h + 1) * HD]
                                nc.tensor.matmul(o1[:], vv,
                                                 e12[:, 0, j, :],
                                                 start=f, stop=l)
                                nc.tensor.matmul(d1[:], dones[:],
                                                 e12[:, 0, j, :],
                                                 start=f, stop=l)
                                nc.tensor.matmul(o2[:], vv,
                                                 e12[:, 1, j, :],
                                                 start=f, stop=l)
                                nc.tensor.matmul(d2[:], dones[:],
                                                 e12[:, 1, j, :],
                                                 start=f, stop=l)

                        for pg in range(KC // 2):
                            s1p = spsum.tile([128, 1024], F32, tag="s")
                            s2p = spsum.tile([128, 1024], F32, tag="s")
                            for j in range(2):
                                kc = pg * 2 + j
                                js = slice(j * 512, (j + 1) * 512)
                                nc.tensor.matmul(
                                    s1p[:, js],
                                    k12[h][0:QD, kc * 128:(kc + 1) * 128],
                                    q12[h][0:QD, qs], start=True, stop=True)
                                nc.tensor.matmul(
                                    s2p[:, js],
                                    k12[h][QD:128, kc * 128:(kc + 1) * 128],
                                    q12[h][QD:128, qs], start=True, stop=True)
                            # e12[:, c, j, :] = exp of chain c, k-chunk 2pg+j
                            e12 = epool.tile([128, 2, 2, 512], BF16,
                                             tag="e12")
                            if mask_trivial:
                                nc.scalar.activation(e12[:, 0, :, :], s1p[:],
                                                     EXP, bias=ebias_t[:],
                                                     scale=SCALE)
                                nc.scalar.activation(e12[:, 1, :, :], s2p[:],
                                                     EXP, bias=ebias_t[:],
                                                     scale=SCALE)
                            else:
                                for j in range(2):
                                    kc = pg * 2 + j
                                    js = slice(j * 512, (j + 1) * 512)
                                    mb = mask_t[:, kc:kc + 1]
                                    nc.scalar.activation(e12[:, 0, j, :],
                                                         s1p[:, js],
                                                         EXP, bias=mb,
                                                         scale=SCALE)
                                    nc.scalar.activation(e12[:, 1, j, :],
                                                         s2p[:, js],
                                                         EXP, bias=mb,
                                                         scale=SCALE)
                            if e_pending is not None:
                                emit_dpv(pg - 1, e_pending)
                            e_pending = e12
                        emit_dpv(KC // 2 - 1, e_pending)

                        R1 = rpool.tile([128, 512], F32, tag="r")
                        nc.vector.reciprocal_approx_fast(R1[:], d1[:])
                        R2 = rpool.tile([128, 512], F32, tag="r")
                        nc.vector.reciprocal_approx_fast(R2[:], d2[:])

                        u1 = tpool.tile([128, 512], F32, tag="u1")
                        nc.vector.tensor_mul(u1[:], o1[:], R1[:])
                        u2 = tpool.tile([128, 512], F32, tag="u2")
                        nc.vector.scalar_tensor_tensor(
                            u2[:], o2[:], lam, R2[:], op0=MULT, op1=MULT)
                        nc.vector.tensor_sub(g_tiles[h][:, qs], u1[:], u2[:])
            # ---------------- Phase 3: output projection ----------------
            with (
                tc.tile_pool(name="wpp2", bufs=1) as wpp2,
                tc.tile_pool(name="yev", bufs=3) as yev,
                tc.tile_pool(name="proj_ps", bufs=4, space="PSUM") as ppsum,
            ):
                for h in (2, 3):
                    w = wpp2.tile([HD, D], BF16, tag=f"wp{h}", name=f"wp{h}")
                    nc.sync.dma_start(
                        w[:], wp[h * HD:(h + 1) * HD, :])
                    wp_tiles.append(w)
                for st in range(S // 128):
                    yt = yev.tile([128, D], F32, tag="yt")
                    for et in range(D // 512):
                        ps = ppsum.tile([128, 512], F32, tag="ps")
                        for h in range(HPC):
                            nc.tensor.matmul(
                                ps[:],
                                g_tiles[h][:, st * 128:(st + 1) * 128],
                                wp_tiles[h][:, et * 512:(et + 1) * 512],
                                start=(h == 0), stop=(h == HPC - 1))
                        nc.vector.tensor_copy(yt[:, et * 512:(et + 1) * 512],
                                              ps[:])
                    nc.sync.dma_start(y[st * 128:(st + 1) * 128, :], yt[:])
            wpp_cm.__exit__(None, None, None)
            gpool_cm.__exit__(None, None, None)

    nc.compile()
    return nc


_PROGRAM_CACHE: dict = {}


def _get_program(lam: float, mask_trivial: bool):
    key = (round(lam, 9), mask_trivial)
    if key not in _PROGRAM_CACHE:
        _PROGRAM_CACHE[key] = build_program(lam, mask_trivial)
    return _PROGRAM_CACHE[key]


def make_in_maps(hidden_states, attention_mask, W_attn, b_attn, W_proj):
    in_maps = []
    for c in range(N_CORES):
        b = c // CPB
        h0 = (c % CPB) * HPC
        xt = np.ascontiguousarray(hidden_states[b].T)
        cols = []
        for h in range(h0, h0 + HPC):
            cols.append(W_attn[:, h * QD:(h + 1) * QD])              # q1
            cols.append(W_attn[:, D // 2 + h * QD:D // 2 + (h + 1) * QD])  # q2
        for h in range(h0, h0 + HPC):
            cols.append(W_attn[:, D + h * QD:D + (h + 1) * QD])      # k1
            cols.append(W_attn[:, D + D // 2 + h * QD:D + D // 2 + (h + 1) * QD])
        wqk = np.ascontiguousarray(np.concatenate(cols, axis=1))
        wv = np.ascontiguousarray(W_attn[:, 2 * D + h0 * HD:2 * D + (h0 + HPC) * HD])
        wpm = np.ascontiguousarray(W_proj[h0 * HD:(h0 + HPC) * HD, :])
        maskb = np.ascontiguousarray(
            ((1.0 - attention_mask[b]) * -10000.0 + EBIAS).reshape(KC, 128)
        ).astype(np.float32)
        in_maps.append({
            "xt": xt.astype(ml_dtypes.bfloat16),
            "wqk": wqk.astype(ml_dtypes.bfloat16),
            "wv": wv.astype(ml_dtypes.bfloat16),
            "wp": wpm.astype(ml_dtypes.bfloat16),
            "maskb": maskb,
        })
    return in_maps


def kernel(hidden_states, attention_mask, W_attn, b_attn, W_proj, b_proj,
           lambda_param, _trace=False):
    hidden_states = np.asarray(hidden_states, np.float32)
    attention_mask = np.asarray(attention_mask, np.float32)
    W_attn = np.asarray(W_attn, np.float32)
    b_attn = np.asarray(b_attn, np.float32)
    W_proj = np.asarray(W_proj, np.float32)
    b_proj = np.asarray(b_proj, np.float32)
    lam = float(np.asarray(lambda_param))

    if np.any(b_attn != 0.0):
        raise NotImplementedError("nonzero b_attn not supported")

    mask_trivial = bool(np.all(attention_mask == 1.0))
    nc = _get_program(lam, mask_trivial)
    in_maps = make_in_maps(hidden_states, attention_mask, W_attn, b_attn,
                           W_proj)
    try:
        res = run_bass_kernel_spmd(nc, in_maps, core_ids=list(range(N_CORES)),
                                   trace=_trace)
    except ModuleNotFoundError:
        res = run_bass_kernel_spmd(nc, in_maps, core_ids=list(range(N_CORES)),
                                   trace=False)

    out = np.empty((B, S, D), np.float32)
    for b in range(B):
        acc = res.results[b * CPB]["y"].astype(np.float32).copy()
        for c in range(b * CPB + 1, (b + 1) * CPB):
            acc += res.results[c]["y"]
        out[b] = acc + b_proj[None, :]
    kernel.last_exec_time_ns = res.exec_time_ns
    if res.instructions_and_trace is not None:
        kernel.last_trace_path = res.instructions_and_trace[1]
    return out


kernel.last_exec_time_ns = None
kernel.last_trace_path = None

